# revision 10
# baseline (speedup 1.0000x reference)
"""Trainium2 Bass kernel for ChronoRotationTransformation.

Computes, per batch row b (B=8192, D=2048):
    u   = (head_r + i*head_i) * (rel_r + i*rel_i)          # complex product
    ab  = sum_d u_r*tail_r - u_i*tail_i                    # == sum rot_r*t_r + rot_i*t_i
    aa  = sum_d u_r^2 + u_i^2                              # == |rot|^2
    bb  = sum_d tail_r^2 + tail_i^2
    out = ab / sqrt(aa*bb)

(The reference's rot = conj(head*rel); rot_r = u_r, rot_i = -u_i, so
ab = rot_r*t_r + rot_i*t_i = u_r*t_r - u_i*t_i and |rot|^2 = |u|^2.)

Sharding: pure data-parallel across 8 NeuronCores, 1024 rows each.
Per core: 8 row-tiles of [128, 2048]. DVE does the 4 cross products,
the two add/subs forming u, and two fused multiply+reduce (ab); ACT
does 4 square+accumulate reductions (aa, bb). Memory-bound target:
~48 MiB HBM reads per core.
"""

import numpy as np

B, D = 8192, 2048
NCORES = 8
BC = B // NCORES            # rows per core
P = 128                     # SBUF partitions
NT = BC // P                # row-tiles per core

IN_NAMES = [
    "head_real", "head_imag",
    "rel_real", "rel_imag",
    "tail_real", "tail_imag",
]

_CACHE = {}


def _emit(tc, ins, out_ap, mybir, repeats=1):
    import concourse.bass as bass  # noqa: F401

    nc = tc.nc
    f32 = mybir.dt.float32
    Alu = mybir.AluOpType
    Act = mybir.ActivationFunctionType

    # DRAM views: [NT, P, D] row-tiles; out as [P, NT] (row = t*128 + p).
    dv = {n: ins[n].rearrange("(t p) d -> t p d", p=P) for n in IN_NAMES}
    out_d = out_ap.rearrange("(t p) -> p t", p=P)

    with (
        tc.tile_pool(name="inp", bufs=2) as inp,
        tc.tile_pool(name="prod", bufs=1) as prod,
        tc.tile_pool(name="upool", bufs=2) as upool,
        tc.tile_pool(name="scr", bufs=1) as scr,
        tc.tile_pool(name="stats", bufs=1) as stats,
    ):
        ab1_s = stats.tile([P, NT], f32, tag="ab1_s")
        ab2_s = stats.tile([P, NT], f32, tag="ab2_s")
        aa1_s = stats.tile([P, NT], f32, tag="aa1_s")
        aa2_s = stats.tile([P, NT], f32, tag="aa2_s")
        bb1_s = stats.tile([P, NT], f32, tag="bb1_s")
        bb2_s = stats.tile([P, NT], f32, tag="bb2_s")

        for _rep in range(repeats):
          for t in range(NT):
            tiles = {}
            for n in IN_NAMES:
                tl = inp.tile([P, D], f32, tag=n)
                nc.sync.dma_start(out=tl[:], in_=dv[n][t])
                tiles[n] = tl
            hr, hi = tiles["head_real"], tiles["head_imag"]
            rr, ri = tiles["rel_real"], tiles["rel_imag"]
            tr, ti = tiles["tail_real"], tiles["tail_imag"]

            m1 = prod.tile([P, D], f32, tag="m1")
            nc.vector.tensor_mul(m1[:], hr[:], rr[:])
            m2 = prod.tile([P, D], f32, tag="m2")
            nc.vector.tensor_mul(m2[:], hi[:], ri[:])
            ur = upool.tile([P, D], f32, tag="ur")
            nc.vector.tensor_sub(ur[:], m1[:], m2[:])
            m3 = prod.tile([P, D], f32, tag="m3")
            nc.vector.tensor_mul(m3[:], hi[:], rr[:])
            m4 = prod.tile([P, D], f32, tag="m4")
            nc.vector.tensor_mul(m4[:], hr[:], ri[:])
            ui = upool.tile([P, D], f32, tag="ui")
            nc.vector.tensor_add(ui[:], m3[:], m4[:])

            # ab = sum(ur*tr) - sum(ui*ti): fused multiply+reduce via
            # scalar_tensor_tensor (out = (in0 op0 scalar) op1 in1,
            # accum_out = sum(out)). tensor_tensor_reduce (native TTR
            # opcode) crashes this terminal's NRT — do not use it.
            so1 = scr.tile([P, D], f32, tag="so1")
            nc.vector.scalar_tensor_tensor(
                out=so1[:], in0=ur[:], scalar=1.0, in1=tr[:],
                op0=Alu.mult, op1=Alu.mult, accum_out=ab1_s[:, t:t + 1],
            )
            so2 = scr.tile([P, D], f32, tag="so2")
            nc.vector.scalar_tensor_tensor(
                out=so2[:], in0=ui[:], scalar=-1.0, in1=ti[:],
                op0=Alu.mult, op1=Alu.mult, accum_out=ab2_s[:, t:t + 1],
            )

            # aa, bb: square+accumulate on ACT.
            for src, dst in (
                (ur, aa1_s), (ui, aa2_s), (tr, bb1_s), (ti, bb2_s),
            ):
                ao = scr.tile([P, D], f32, tag="ao")
                nc.scalar.activation(
                    out=ao[:], in_=src[:], func=Act.Square,
                    accum_out=dst[:, t:t + 1],
                )

        # Final combine on [P, NT] (tiny).
        fin = {}
        def ftile(name):
            tl = stats.tile([P, NT], f32, tag=name)
            fin[name] = tl
            return tl

        ab = ftile("ab"); nc.vector.tensor_add(ab[:], ab1_s[:], ab2_s[:])
        aa = ftile("aa"); nc.vector.tensor_add(aa[:], aa1_s[:], aa2_s[:])
        bb = ftile("bb"); nc.vector.tensor_add(bb[:], bb1_s[:], bb2_s[:])
        pp = ftile("pp"); nc.vector.tensor_mul(pp[:], aa[:], bb[:])
        # sqrt on ACT is low precision (up to ~65536 ULP budget); refine
        # with two Newton iterations  r <- 0.5*(r + p/r)  using the
        # bit-exact DVE reciprocal.
        r = ftile("r0"); nc.scalar.activation(out=r[:], in_=pp[:], func=Act.Sqrt)
        for it in range(2):
            q = ftile(f"q{it}"); nc.vector.reciprocal(q[:], r[:])
            pq = ftile(f"pq{it}"); nc.vector.tensor_mul(pq[:], pp[:], q[:])
            s = ftile(f"s{it}"); nc.vector.tensor_add(s[:], r[:], pq[:])
            r = ftile(f"r{it + 1}"); nc.vector.tensor_scalar_mul(r[:], s[:], 0.5)
        inv = ftile("inv"); nc.vector.reciprocal(inv[:], r[:])
        score = ftile("score"); nc.vector.tensor_mul(score[:], ab[:], inv[:])
        nc.sync.dma_start(out=out_d, in_=score[:])


def _build(repeats=1):
    key = ("nc", repeats)
    if key in _CACHE:
        return _CACHE[key]
    import concourse.tile as tile
    from concourse import bacc, mybir

    # NOTE: num_devices is deliberately NOT set — it enables collective
    # global-comm setup that breaks plain SPMD input binding under the
    # axon/PJRT path (outputs come back as garbage).
    nc = bacc.Bacc(
        "TRN2",
        target_bir_lowering=False,
        debug=False,
    )
    ins = {
        n: nc.dram_tensor(n, [BC, D], mybir.dt.float32, kind="ExternalInput").ap()
        for n in IN_NAMES
    }
    out = nc.dram_tensor("out", [BC], mybir.dt.float32, kind="ExternalOutput").ap()
    with tile.TileContext(nc) as tc:
        _emit(tc, ins, out, mybir, repeats=repeats)
    nc.compile()
    _CACHE[key] = nc
    return nc


def run(inputs, trace=False, **kwargs):
    """Run on 8 cores; returns (full_output, BassKernelResults)."""
    from concourse.bass_utils import run_bass_kernel_spmd

    nc = _build()
    core_ids = list(range(NCORES))
    in_maps = []
    for c in range(NCORES):
        sl = slice(c * BC, (c + 1) * BC)
        in_maps.append(
            {n: np.ascontiguousarray(inputs[n][sl], dtype=np.float32)
             for n in IN_NAMES}
        )
    # The terminal occasionally reports the accelerator unrecoverable
    # (e.g. poisoned by an earlier crashed run); a fresh attempt after a
    # short wait triggers recovery.
    last_exc = None
    for attempt in range(4):
        try:
            res = run_bass_kernel_spmd(nc, in_maps, core_ids, trace=trace, **kwargs)
            break
        except Exception as e:  # noqa: BLE001
            last_exc = e
            if attempt == 3:
                raise
            import time as _time
            _time.sleep(15 * (attempt + 1))
    out = np.concatenate([res.results[c]["out"] for c in range(NCORES)])
    return out.astype(np.float32), res


def kernel(**inputs):
    out, _ = run(inputs)
    return out
